# revision 9
# baseline (speedup 1.0000x reference)
"""DiagSSMBlock Trainium2 kernel.

h_t = sum_{k=0..t} a^k * (B^T x_{t-k})  ==  h_t = a * h_{t-1} + s_t, s = B^T x^T.

Strategy: shard T across the 8 cores (1024 steps each + 32-step halo; |a| <=
sqrt(2/1024) ~ 0.044 so a^32 < 1e-43 == 0 in fp32, making slabs exactly
independent).  Matmul operands are cast to fp16 on host (tolerance is 2e-2;
fp16 keeps rel err ~1e-3) which halves input HBM traffic and enables FWL on
the PE weight path.  Host pre-packs both operands so every DMA is a single
large 2D contiguous transfer:
  xt[p, ni*KQ*CH + kq*CH + c] = x_padded.T[kq*128+p, ni*CH+c]
  b [p, kq*H + col]           = b_mat[kq*128+p, col]

Per core: inputs stream over both HWDGE queues (sync + scalar), balanced
~50/50 and ordered so the PE can start accumulating early and PSUM chains
retire before the full input set lands: x chunk0 halves first on both queues,
then b kq-slabs 0-3 (sync) / 4-7 (scalar), then x chunk1/2 halves.  The
contraction consumes kq in arrival order (0,4,1,5,...).  Scan runs on DVE
per 128-channel group (fp32 internal state, fp16 h tiles / fp16 stores,
upcast on host); each chunk's slab is stored to HBM immediately after its
scan, alternating queues.  Warm-up matmuls lift the PE HAM clock-gate to
2.4 GHz during the initial DMA ramp.
"""

import sys

if "/opt/trn_rl_repo" not in sys.path:
    sys.path.insert(0, "/opt/trn_rl_repo")

import numpy as np

T, H = 8192, 1024
NC = 8
P = 128
T_LOC = T // NC            # 1024 output timesteps per core
HALO = 32                  # scan warmup; a^32 == 0 in fp32
W = T_LOC + HALO           # 1056
CH = 352                   # psum chunk width (3 chunks of 352 = 1056)
NCHUNK = W // CH
KQ = H // P                # 8 contraction chunks
G = H // P                 # 8 channel groups
N_WARM = 10                # dummy matmuls to lift the HAM clock gate

KQ_ORDER = [0, 4, 1, 5, 2, 6, 3, 7]   # kq consumption order == DMA arrival

_state = {}


def _build_nc():
    import concourse.tile as tile
    from concourse import bacc, mybir

    mm_dt = mybir.dt.float16
    f32 = mybir.dt.float32

    nc = bacc.Bacc("TRN2", target_bir_lowering=False, debug=False, num_devices=NC)
    xt_e = nc.dram_tensor("xt", [P, NCHUNK * KQ * CH], mm_dt, kind="ExternalInput").ap()
    b_e = nc.dram_tensor("b", [P, KQ * H], mm_dt, kind="ExternalInput").ap()
    av_e = nc.dram_tensor("av", [P, G], f32, kind="ExternalInput").ap()
    out_e = nc.dram_tensor("out", [H, T_LOC], mm_dt, kind="ExternalOutput").ap()
    flush_e = nc.dram_tensor("warm_flush", [P, 1], f32).ap()

    XC = KQ * CH  # 2816 columns per x chunk
    XH = XC // 2  # half-chunk: kq 0-3 or 4-7

    with tile.TileContext(nc) as tc:
        with (
            tc.tile_pool(name="consts", bufs=1) as consts,
            tc.tile_pool(name="bpool", bufs=1) as bpool,
            tc.tile_pool(name="xpool", bufs=1) as xpool,
            tc.tile_pool(name="hpool", bufs=1) as hpool,
            tc.tile_pool(name="pspool", bufs=7, space="PSUM") as pspool,
            tc.tile_pool(name="warmps", bufs=1, space="PSUM") as warmps,
        ):
            # --- input DMA issue, balanced across the two HWDGE queues ---
            av_sb = consts.tile([P, G], f32, tag="av")
            nc.sync.dma_start(av_sb[:], av_e[:])

            x_sb = [
                xpool.tile([P, XC], mm_dt, tag=f"x{ni}", name=f"x{ni}")
                for ni in range(NCHUNK)
            ]
            b_sb = [
                bpool.tile([P, H], mm_dt, tag=f"b{kq}", name=f"b{kq}")
                for kq in range(KQ)
            ]

            # chunk0 halves first on both queues (kq 0-3 on sync, 4-7 on scalar)
            nc.sync.dma_start(x_sb[0][:, 0:XH], xt_e[:, 0:XH])
            nc.scalar.dma_start(x_sb[0][:, XH:XC], xt_e[:, XH:XC])
            # b slabs: 0-3 on sync, 4-7 on scalar
            for kq in range(4):
                nc.sync.dma_start(b_sb[kq][:], b_e[:, kq * H : (kq + 1) * H])
            for kq in range(4, 8):
                nc.scalar.dma_start(b_sb[kq][:], b_e[:, kq * H : (kq + 1) * H])
            # remaining x chunks, halves split across queues
            for ni in (1, 2):
                o = ni * XC
                nc.sync.dma_start(x_sb[ni][:, 0:XH], xt_e[:, o : o + XH])
                nc.scalar.dma_start(x_sb[ni][:, XH:XC], xt_e[:, o + XH : o + XC])

            # --- PE warm-up (HAM clock-gate lift) during the DMA ramp ---
            warm_sb = consts.tile([P, CH], mm_dt, tag="warm")
            nc.vector.memset(warm_sb[:], 0.0)
            wps = warmps.tile([P, CH], f32)
            for i in range(N_WARM):
                nc.tensor.matmul(
                    wps[:],
                    warm_sb[:, 0:P],
                    warm_sb[:],
                    start=(i == 0),
                    stop=(i == N_WARM - 1),
                )

            # --- a broadcast tiles (DVE) ---
            ones = consts.tile([P, CH], f32, tag="ones")
            nc.vector.memset(ones[:], 1.0)
            a_bc = []
            for g in range(G):
                t = consts.tile([P, CH], f32, tag=f"abc{g}", name=f"abc{g}")
                nc.vector.tensor_scalar_mul(t[:], ones[:], av_sb[:, g : g + 1])
                a_bc.append(t)

            # --- matmul chains + scans + stores ---
            store_eng = [nc.sync, nc.scalar]
            n_store = 0
            for g in range(G):
                h_t = hpool.tile([P, W], mm_dt, tag=f"h{g}")
                for ni in range(NCHUNK):
                    n0 = ni * CH
                    ps = pspool.tile([P, CH], f32)
                    for j, kq in enumerate(KQ_ORDER):
                        nc.tensor.matmul(
                            ps[:],
                            b_sb[kq][:, g * P : (g + 1) * P],
                            x_sb[ni][:, kq * CH : (kq + 1) * CH],
                            start=(j == 0),
                            stop=(j == KQ - 1),
                        )
                    init = 0.0 if ni == 0 else h_t[:, n0 - 1 : n0]
                    nc.vector.tensor_tensor_scan(
                        h_t[:, n0 : n0 + CH],
                        a_bc[g][:],
                        ps[:],
                        init,
                        op0=mybir.AluOpType.mult,
                        op1=mybir.AluOpType.add,
                    )
                    # store this chunk's finished slab right away
                    c0 = max(n0, HALO)
                    store_eng[n_store % 2].dma_start(
                        out_e[g * P : (g + 1) * P, c0 - HALO : n0 + CH - HALO],
                        h_t[:, c0 : n0 + CH],
                    )
                    n_store += 1

            # keep the warm-up matmuls live (flush wps so they aren't dead)
            flush_sb = consts.tile([P, 1], f32, tag="flush")
            nc.vector.tensor_copy(flush_sb[:], wps[:, 0:1])
            nc.gpsimd.dma_start(flush_e[:], flush_sb[:])

    nc.compile()
    return nc


def _get_nc():
    if "nc" not in _state:
        _state["nc"] = _build_nc()
    return _state["nc"]


def _shard_inputs(x_seq, a_diag, b_mat):
    x = np.asarray(x_seq, dtype=np.float32)
    a = np.asarray(a_diag, dtype=np.float32)
    b = np.asarray(b_mat, dtype=np.float32)
    x_pad = np.concatenate([np.zeros((HALO, H), np.float32), x], axis=0)
    xT = x_pad.T  # [H, T + HALO]
    # b packed: b_host[p, kq*H + col] = b[kq*128+p, col]
    b_host = np.ascontiguousarray(
        b.reshape(KQ, P, H).transpose(1, 0, 2).reshape(P, KQ * H)
    ).astype(np.float16)
    av = np.ascontiguousarray(a.reshape(G, P).T)  # [P, G]
    in_maps = []
    for i in range(NC):
        slab = xT[:, i * T_LOC : i * T_LOC + W]  # [H, W]
        # x packed: x_host[p, ni*KQ*CH + kq*CH + c] = slab[kq*128+p, ni*CH+c]
        xh = (
            slab.reshape(KQ, P, NCHUNK, CH)
            .transpose(1, 2, 0, 3)
            .reshape(P, NCHUNK * KQ * CH)
        )
        in_maps.append(
            {
                "xt": np.ascontiguousarray(xh).astype(np.float16),
                "b": b_host,
                "av": av,
            }
        )
    return in_maps


def kernel(x_seq, a_diag, b_mat):
    from concourse.bass_utils import run_bass_kernel_spmd

    nc = _get_nc()
    in_maps = _shard_inputs(x_seq, a_diag, b_mat)
    res = run_bass_kernel_spmd(nc, in_maps, list(range(NC)))
    _state["last_result"] = res
    out = np.concatenate(
        [np.asarray(res.results[i]["out"]).astype(np.float32).T for i in range(NC)],
        axis=0,
    )
    return out


# revision 11
# speedup vs baseline: 1.0754x; 1.0754x over previous
"""DiagSSMBlock Trainium2 kernel.

h_t = sum_{k=0..t} a^k * (B^T x_{t-k})  ==  h_t = a * h_{t-1} + s_t, s = B^T x^T.

Strategy: shard T across the 8 cores (1024 steps each + 32-step halo; |a| <=
sqrt(2/1024) ~ 0.044 so a^32 < 1e-43 == 0 in fp32, making slabs exactly
independent).  Matmul operands are cast to fp16 on host (tolerance is 2e-2;
fp16 keeps rel err ~1e-3) which halves input HBM traffic and enables FWL on
the PE weight path.  Host pre-packs both operands so every DMA is a single
large 2D contiguous transfer:
  xt[p, ni*KQ*CH + kq*CH + c] = x_padded.T[kq*128+p, ni*CH+c]
  b [p, kq*H + col]           = b_mat[kq*128+p, col]

Per core: inputs stream over both HWDGE queues (sync + scalar), balanced
~50/50 and ordered so the PE can start accumulating early and PSUM chains
retire before the full input set lands: x chunk0 halves first on both queues,
then b kq-slabs 0-3 (sync) / 4-7 (scalar), then x chunk1/2 halves.  The
contraction consumes kq in arrival order (0,4,1,5,...).  Scan runs on DVE
per 128-channel group (fp32 internal state, fp16 h tiles / fp16 stores,
upcast on host); each chunk's slab is stored to HBM immediately after its
scan, alternating queues.  Warm-up matmuls lift the PE HAM clock-gate to
2.4 GHz during the initial DMA ramp.
"""

import sys

if "/opt/trn_rl_repo" not in sys.path:
    sys.path.insert(0, "/opt/trn_rl_repo")

import numpy as np

T, H = 8192, 1024
NC = 8
P = 128
T_LOC = T // NC            # 1024 output timesteps per core
HALO = 32                  # scan warmup; a^32 == 0 in fp32
W = T_LOC + HALO           # 1056
CH = 352                   # psum chunk width (3 chunks of 352 = 1056)
NCHUNK = W // CH
KQ = H // P                # 8 contraction chunks
G = H // P                 # 8 channel groups
N_WARM = 10                # dummy matmuls to lift the HAM clock gate

KQ_ORDER = [0, 4, 1, 5, 2, 6, 3, 7]   # kq consumption order == DMA arrival

_state = {}


def _build_nc():
    import concourse.tile as tile
    from concourse import bacc, mybir

    mm_dt = mybir.dt.float16
    f32 = mybir.dt.float32

    nc = bacc.Bacc("TRN2", target_bir_lowering=False, debug=False, num_devices=NC)
    xt_e = nc.dram_tensor("xt", [P, NCHUNK * KQ * CH], mm_dt, kind="ExternalInput").ap()
    b_e = nc.dram_tensor("b", [P, KQ * H], mm_dt, kind="ExternalInput").ap()
    av_e = nc.dram_tensor("av", [P, G], f32, kind="ExternalInput").ap()
    out_e = nc.dram_tensor("out", [H, T_LOC], mm_dt, kind="ExternalOutput").ap()
    flush_e = nc.dram_tensor("warm_flush", [P, 1], f32).ap()

    XC = KQ * CH  # 2816 columns per x chunk
    XH = XC // 2  # half-chunk: kq 0-3 or 4-7

    with tile.TileContext(nc) as tc:
        with (
            tc.tile_pool(name="consts", bufs=1) as consts,
            tc.tile_pool(name="bpool", bufs=1) as bpool,
            tc.tile_pool(name="xpool", bufs=1) as xpool,
            tc.tile_pool(name="hpool", bufs=1) as hpool,
            tc.tile_pool(name="pspool", bufs=7, space="PSUM") as pspool,
            tc.tile_pool(name="warmps", bufs=1, space="PSUM") as warmps,
        ):
            # --- input DMA issue, balanced across the two HWDGE queues ---
            av_sb = consts.tile([P, G], f32, tag="av")
            nc.sync.dma_start(av_sb[:], av_e[:])

            x_sb = [
                xpool.tile([P, XC], mm_dt, tag=f"x{ni}", name=f"x{ni}")
                for ni in range(NCHUNK)
            ]
            b_sb = [
                bpool.tile([P, H], mm_dt, tag=f"b{kq}", name=f"b{kq}")
                for kq in range(KQ)
            ]

            # B first (PE FIFO blocks on missing b slabs; x2 is consumed last
            # and tolerates the chip-wide HBM contention window), x0 between
            # the b pairs so the first chains can start.
            nc.sync.dma_start(b_sb[0][:], b_e[:, 0:H])
            nc.scalar.dma_start(b_sb[4][:], b_e[:, 4 * H : 5 * H])
            nc.sync.dma_start(b_sb[1][:], b_e[:, H : 2 * H])
            nc.scalar.dma_start(b_sb[5][:], b_e[:, 5 * H : 6 * H])
            nc.sync.dma_start(x_sb[0][:, 0:XH], xt_e[:, 0:XH])
            nc.scalar.dma_start(x_sb[0][:, XH:XC], xt_e[:, XH:XC])
            nc.sync.dma_start(b_sb[2][:], b_e[:, 2 * H : 3 * H])
            nc.scalar.dma_start(b_sb[6][:], b_e[:, 6 * H : 7 * H])
            nc.sync.dma_start(b_sb[3][:], b_e[:, 3 * H : 4 * H])
            nc.scalar.dma_start(b_sb[7][:], b_e[:, 7 * H : 8 * H])
            for ni in (1, 2):
                o = ni * XC
                nc.sync.dma_start(x_sb[ni][:, 0:XH], xt_e[:, o : o + XH])
                nc.scalar.dma_start(x_sb[ni][:, XH:XC], xt_e[:, o + XH : o + XC])

            # --- PE warm-up (HAM clock-gate lift) during the DMA ramp ---
            warm_sb = consts.tile([P, CH], mm_dt, tag="warm")
            nc.vector.memset(warm_sb[:], 0.0)
            wps = warmps.tile([P, CH], f32)
            for i in range(N_WARM):
                nc.tensor.matmul(
                    wps[:],
                    warm_sb[:, 0:P],
                    warm_sb[:],
                    start=(i == 0),
                    stop=(i == N_WARM - 1),
                )

            # --- a broadcast tiles (DVE) ---
            ones = consts.tile([P, CH], f32, tag="ones")
            nc.vector.memset(ones[:], 1.0)
            a_bc = []
            for g in range(G):
                t = consts.tile([P, CH], f32, tag=f"abc{g}", name=f"abc{g}")
                nc.vector.tensor_scalar_mul(t[:], ones[:], av_sb[:, g : g + 1])
                a_bc.append(t)

            # --- matmul chains + scans + stores ---
            store_eng = [nc.sync, nc.scalar]
            n_store = 0
            for g in range(G):
                h_t = hpool.tile([P, W], mm_dt, tag=f"h{g}")
                for ni in range(NCHUNK):
                    n0 = ni * CH
                    ps = pspool.tile([P, CH], f32)
                    for j, kq in enumerate(KQ_ORDER):
                        nc.tensor.matmul(
                            ps[:],
                            b_sb[kq][:, g * P : (g + 1) * P],
                            x_sb[ni][:, kq * CH : (kq + 1) * CH],
                            start=(j == 0),
                            stop=(j == KQ - 1),
                        )
                    init = 0.0 if ni == 0 else h_t[:, n0 - 1 : n0]
                    nc.vector.tensor_tensor_scan(
                        h_t[:, n0 : n0 + CH],
                        a_bc[g][:],
                        ps[:],
                        init,
                        op0=mybir.AluOpType.mult,
                        op1=mybir.AluOpType.add,
                    )
                    # store this chunk's finished slab right away
                    c0 = max(n0, HALO)
                    store_eng[n_store % 2].dma_start(
                        out_e[g * P : (g + 1) * P, c0 - HALO : n0 + CH - HALO],
                        h_t[:, c0 : n0 + CH],
                    )
                    n_store += 1
                    if g < 2:
                        # keep-warm fillers: hold the HAM clock-gate open
                        # through input-DMA hiccups in the early phase
                        for i in range(2):
                            nc.tensor.matmul(
                                wps[:],
                                warm_sb[:, 0:P],
                                warm_sb[:],
                                start=(i == 0),
                                stop=(i == 1),
                            )

            # keep the warm-up matmuls live (flush wps so they aren't dead)
            flush_sb = consts.tile([P, 1], f32, tag="flush")
            nc.vector.tensor_copy(flush_sb[:], wps[:, 0:1])
            nc.gpsimd.dma_start(flush_e[:], flush_sb[:])

    nc.compile()
    return nc


def _get_nc():
    if "nc" not in _state:
        _state["nc"] = _build_nc()
    return _state["nc"]


def _shard_inputs(x_seq, a_diag, b_mat):
    x = np.asarray(x_seq, dtype=np.float32)
    a = np.asarray(a_diag, dtype=np.float32)
    b = np.asarray(b_mat, dtype=np.float32)
    x_pad = np.concatenate([np.zeros((HALO, H), np.float32), x], axis=0)
    xT = x_pad.T  # [H, T + HALO]
    # b packed: b_host[p, kq*H + col] = b[kq*128+p, col]
    b_host = np.ascontiguousarray(
        b.reshape(KQ, P, H).transpose(1, 0, 2).reshape(P, KQ * H)
    ).astype(np.float16)
    av = np.ascontiguousarray(a.reshape(G, P).T)  # [P, G]
    in_maps = []
    for i in range(NC):
        slab = xT[:, i * T_LOC : i * T_LOC + W]  # [H, W]
        # x packed: x_host[p, ni*KQ*CH + kq*CH + c] = slab[kq*128+p, ni*CH+c]
        xh = (
            slab.reshape(KQ, P, NCHUNK, CH)
            .transpose(1, 2, 0, 3)
            .reshape(P, NCHUNK * KQ * CH)
        )
        in_maps.append(
            {
                "xt": np.ascontiguousarray(xh).astype(np.float16),
                "b": b_host,
                "av": av,
            }
        )
    return in_maps


def kernel(x_seq, a_diag, b_mat):
    from concourse.bass_utils import run_bass_kernel_spmd

    nc = _get_nc()
    in_maps = _shard_inputs(x_seq, a_diag, b_mat)
    res = run_bass_kernel_spmd(nc, in_maps, list(range(NC)))
    _state["last_result"] = res
    out = np.concatenate(
        [np.asarray(res.results[i]["out"]).astype(np.float32).T for i in range(NC)],
        axis=0,
    )
    return out


# revision 13
# speedup vs baseline: 1.1736x; 1.0913x over previous
"""DiagSSMBlock Trainium2 kernel.

h_t = sum_{k=0..t} a^k * (B^T x_{t-k})  ==  h_t = a * h_{t-1} + s_t, s = B^T x^T.

Strategy: shard T across the 8 cores (1024 steps each + 32-step halo; |a| <=
sqrt(2/1024) ~ 0.044 so a^32 < 1e-43 == 0 in fp32, making slabs exactly
independent).  Matmul operands are cast to fp16 on host (tolerance is 2e-2;
fp16 keeps rel err ~1e-3) which halves input HBM traffic and enables FWL on
the PE weight path.  Host pre-packs both operands so every DMA is a single
large 2D contiguous transfer:
  xt[p, ni*KQ*CH + kq*CH + c] = x_padded.T[kq*128+p, ni*CH+c]
  b [p, kq*H + col]           = b_mat[kq*128+p, col]

Per core: inputs stream over both HWDGE queues (sync + scalar), balanced
~50/50 and ordered so the PE can start accumulating early and PSUM chains
retire before the full input set lands: x chunk0 halves first on both queues,
then b kq-slabs 0-3 (sync) / 4-7 (scalar), then x chunk1/2 halves.  The
contraction consumes kq in arrival order (0,4,1,5,...).  Scan runs on DVE
per 128-channel group (fp32 internal state, fp16 h tiles / fp16 stores,
upcast on host); each chunk's slab is stored to HBM immediately after its
scan, alternating queues.  Warm-up matmuls lift the PE HAM clock-gate to
2.4 GHz during the initial DMA ramp.
"""

import sys

if "/opt/trn_rl_repo" not in sys.path:
    sys.path.insert(0, "/opt/trn_rl_repo")

import numpy as np

T, H = 8192, 1024
NC = 8
P = 128
T_LOC = T // NC            # 1024 output timesteps per core
HALO = 32                  # scan warmup; a^32 == 0 in fp32
W = T_LOC + HALO           # 1056
CH = 352                   # psum chunk width (3 chunks of 352 = 1056)
NCHUNK = W // CH
KQ = H // P                # 8 contraction chunks
G = H // P                 # 8 channel groups
N_WARM = 10                # dummy matmuls to lift the HAM clock gate

KQ_ORDER = [0, 4, 1, 5, 2, 6, 3, 7]   # kq consumption order == DMA arrival

_state = {}


def _build_nc():
    import concourse.tile as tile
    from concourse import bacc, mybir

    mm_dt = mybir.dt.float16
    f32 = mybir.dt.float32

    nc = bacc.Bacc("TRN2", target_bir_lowering=False, debug=False, num_devices=NC)
    xt_e = nc.dram_tensor("xt", [P, NCHUNK * KQ * CH], mm_dt, kind="ExternalInput").ap()
    b_e = nc.dram_tensor("b", [P, KQ * H], mm_dt, kind="ExternalInput").ap()
    av_e = nc.dram_tensor("av", [P, G], f32, kind="ExternalInput").ap()
    out_e = nc.dram_tensor("out", [H, T_LOC], mm_dt, kind="ExternalOutput").ap()
    flush_e = nc.dram_tensor("warm_flush", [P, 1], f32).ap()

    XC = KQ * CH  # 2816 columns per x chunk
    XH = XC // 2  # half-chunk: kq 0-3 or 4-7

    with tile.TileContext(nc) as tc:
        with (
            tc.tile_pool(name="consts", bufs=1) as consts,
            tc.tile_pool(name="bpool", bufs=1) as bpool,
            tc.tile_pool(name="xpool", bufs=1) as xpool,
            tc.tile_pool(name="hpool", bufs=1) as hpool,
            tc.tile_pool(name="pspool", bufs=7, space="PSUM") as pspool,
            tc.tile_pool(name="warmps", bufs=1, space="PSUM") as warmps,
        ):
            # --- input DMA issue, balanced across the two HWDGE queues ---
            av_sb = consts.tile([P, G], f32, tag="av")
            nc.sync.dma_start(av_sb[:], av_e[:])

            x_sb = [
                xpool.tile([P, XC], mm_dt, tag=f"x{ni}", name=f"x{ni}")
                for ni in range(NCHUNK)
            ]
            b_sb = [
                bpool.tile([P, H], mm_dt, tag=f"b{kq}", name=f"b{kq}")
                for kq in range(KQ)
            ]

            # B-priority with quartered x0 interleaved: the PE FIFO blocks on
            # missing b slabs, so all of B lands in the first ~7us; x1/x2 are
            # consumed late (ni-major chain order) and tolerate the chip-wide
            # HBM contention window when all 8 cores load simultaneously.
            XQ = XC // 4
            nc.sync.dma_start(b_sb[0][:], b_e[:, 0:H])
            nc.scalar.dma_start(b_sb[4][:], b_e[:, 4 * H : 5 * H])
            nc.sync.dma_start(x_sb[0][:, 0:XQ], xt_e[:, 0:XQ])
            nc.scalar.dma_start(x_sb[0][:, 2 * XQ : 3 * XQ], xt_e[:, 2 * XQ : 3 * XQ])
            nc.sync.dma_start(b_sb[1][:], b_e[:, H : 2 * H])
            nc.scalar.dma_start(b_sb[5][:], b_e[:, 5 * H : 6 * H])
            nc.sync.dma_start(x_sb[0][:, XQ : 2 * XQ], xt_e[:, XQ : 2 * XQ])
            nc.scalar.dma_start(x_sb[0][:, 3 * XQ : 4 * XQ], xt_e[:, 3 * XQ : 4 * XQ])
            nc.sync.dma_start(b_sb[2][:], b_e[:, 2 * H : 3 * H])
            nc.scalar.dma_start(b_sb[6][:], b_e[:, 6 * H : 7 * H])
            nc.sync.dma_start(b_sb[3][:], b_e[:, 3 * H : 4 * H])
            nc.scalar.dma_start(b_sb[7][:], b_e[:, 7 * H : 8 * H])
            for ni in (1, 2):
                o = ni * XC
                nc.sync.dma_start(x_sb[ni][:, 0:XH], xt_e[:, o : o + XH])
                nc.scalar.dma_start(x_sb[ni][:, XH:XC], xt_e[:, o + XH : o + XC])

            # --- PE warm-up (HAM clock-gate lift) during the DMA ramp ---
            warm_sb = consts.tile([P, CH], mm_dt, tag="warm")
            nc.vector.memset(warm_sb[:], 0.0)
            wps = warmps.tile([P, CH], f32)
            for i in range(N_WARM):
                nc.tensor.matmul(
                    wps[:],
                    warm_sb[:, 0:P],
                    warm_sb[:],
                    start=(i == 0),
                    stop=(i == N_WARM - 1),
                )

            # --- a broadcast tiles (DVE) ---
            ones = consts.tile([P, CH], f32, tag="ones")
            nc.vector.memset(ones[:], 1.0)
            a_bc = []
            for g in range(G):
                t = consts.tile([P, CH], f32, tag=f"abc{g}", name=f"abc{g}")
                nc.vector.tensor_scalar_mul(t[:], ones[:], av_sb[:, g : g + 1])
                a_bc.append(t)

            # --- matmul chains + scans + stores (ni-major: x1/x2 needed late) ---
            store_eng = [nc.sync, nc.scalar]
            h_sb = [
                hpool.tile([P, W], mm_dt, tag=f"h{g}", name=f"h{g}")
                for g in range(G)
            ]
            n_store = 0
            for ni in range(NCHUNK):
                n0 = ni * CH
                for g in range(G):
                    h_t = h_sb[g]
                    ps = pspool.tile([P, CH], f32)
                    for j, kq in enumerate(KQ_ORDER):
                        nc.tensor.matmul(
                            ps[:],
                            b_sb[kq][:, g * P : (g + 1) * P],
                            x_sb[ni][:, kq * CH : (kq + 1) * CH],
                            start=(j == 0),
                            stop=(j == KQ - 1),
                        )
                    init = 0.0 if ni == 0 else h_t[:, n0 - 1 : n0]
                    nc.vector.tensor_tensor_scan(
                        h_t[:, n0 : n0 + CH],
                        a_bc[g][:],
                        ps[:],
                        init,
                        op0=mybir.AluOpType.mult,
                        op1=mybir.AluOpType.add,
                    )
                    # store this chunk's finished slab right away
                    c0 = max(n0, HALO)
                    store_eng[n_store % 2].dma_start(
                        out_e[g * P : (g + 1) * P, c0 - HALO : n0 + CH - HALO],
                        h_t[:, c0 : n0 + CH],
                    )
                    n_store += 1
                    if ni == 0 and g < 2:
                        # keep-warm fillers: hold the HAM clock-gate open
                        # through input-DMA hiccups in the early phase
                        for i in range(2):
                            nc.tensor.matmul(
                                wps[:],
                                warm_sb[:, 0:P],
                                warm_sb[:],
                                start=(i == 0),
                                stop=(i == 1),
                            )

            # keep the warm-up matmuls live (flush wps so they aren't dead)
            flush_sb = consts.tile([P, 1], f32, tag="flush")
            nc.vector.tensor_copy(flush_sb[:], wps[:, 0:1])
            nc.gpsimd.dma_start(flush_e[:], flush_sb[:])

    nc.compile()
    return nc


def _get_nc():
    if "nc" not in _state:
        _state["nc"] = _build_nc()
    return _state["nc"]


def _shard_inputs(x_seq, a_diag, b_mat):
    x = np.asarray(x_seq, dtype=np.float32)
    a = np.asarray(a_diag, dtype=np.float32)
    b = np.asarray(b_mat, dtype=np.float32)
    x_pad = np.concatenate([np.zeros((HALO, H), np.float32), x], axis=0)
    xT = x_pad.T  # [H, T + HALO]
    # b packed: b_host[p, kq*H + col] = b[kq*128+p, col]
    b_host = np.ascontiguousarray(
        b.reshape(KQ, P, H).transpose(1, 0, 2).reshape(P, KQ * H)
    ).astype(np.float16)
    av = np.ascontiguousarray(a.reshape(G, P).T)  # [P, G]
    in_maps = []
    for i in range(NC):
        slab = xT[:, i * T_LOC : i * T_LOC + W]  # [H, W]
        # x packed: x_host[p, ni*KQ*CH + kq*CH + c] = slab[kq*128+p, ni*CH+c]
        xh = (
            slab.reshape(KQ, P, NCHUNK, CH)
            .transpose(1, 2, 0, 3)
            .reshape(P, NCHUNK * KQ * CH)
        )
        in_maps.append(
            {
                "xt": np.ascontiguousarray(xh).astype(np.float16),
                "b": b_host,
                "av": av,
            }
        )
    return in_maps


def kernel(x_seq, a_diag, b_mat):
    from concourse.bass_utils import run_bass_kernel_spmd

    nc = _get_nc()
    in_maps = _shard_inputs(x_seq, a_diag, b_mat)
    res = run_bass_kernel_spmd(nc, in_maps, list(range(NC)))
    _state["last_result"] = res
    out = np.concatenate(
        [np.asarray(res.results[i]["out"]).astype(np.float32).T for i in range(NC)],
        axis=0,
    )
    return out
